# revision 18
# baseline (speedup 1.0000x reference)
"""Trainium2 Bass kernel for nn_BidirectionalLayerFeatCosine (retrieval_knn).

Strategy: shard the 4096 query points across 8 NeuronCores (512 each); keys
are replicated.  Each core runs an identical SPMD program over 4 combos
(2 crosses x 2 batches):
  - key prep: khat = knn / ||knn|| (ACT square, Pool partition all-reduce,
    ACT sqrt + DVE recip on a [32,128] DRAM-reshape, Pool multiply)
  - akv/cq linear layers on PE in bf16 (host-cast inputs); MLP in f32r.
    Score matmuls stay strict fp32: fp32r matmuls adjacent to them corrupt
    fp32 matmul precision on HW (observed), bf16/f32r-MLP do not.
  - scores stream through PSUM in [128,1024] chunks; DVE max8/max_index8
    read PSUM directly (no PSUM->SBUF copies); queries stay UNnormalized
    for cosine (per-query positive scale is rank-monotone in fp32)
  - exact global top-8 via per-chunk top-8 + one gpsimd local_scatter pair
    (per-partition scatter) to resolve winner indices per query
  - GpSimd ap_gather pulls neighbor features, Pool broadcast-add of cq,
    PE 64x64 MLP, ACT leaky-relu, DVE windowed-max pool (bf16)
"""
import sys

for _p in ('/opt/trn_rl_repo',):
    if _p not in sys.path:
        sys.path.insert(0, _p)

import numpy as np
import concourse.bass as bass
import concourse.bass_isa as bass_isa
import concourse.tile as tile
from concourse import bacc, mybir

F32 = mybir.dt.float32
F32R = mybir.dt.float32r
BF16 = mybir.dt.bfloat16
I16 = mybir.dt.int16
U16 = mybir.dt.uint16
AF = mybir.ActivationFunctionType
ALU = mybir.AluOpType
AXX = mybir.AxisListType.X

B, N, C, NS = 2, 4096, 64, 16
NCORES = 8
LEAKY = 0.1
EPS = 1e-8
CHW = 1024            # score chunk width (2 PSUM banks)
NCH = N // CHW        # chunks per metric row (4)
MLP_DT = F32R


def _neg_sq_row(tc, pools, pc_ap, length, blk3, id128, dram_pool, tag):
    """-(x^2+y^2+z^2) per point of pc_ap [3, length] (DRAM) -> DRAM scratch
    [nbs, 128] whose flat layout == [length] row.  Returns the scratch tile.
    Identical numerics to the proven baseline (PE fp32 + ACT)."""
    nc = tc.nc
    nbs = length // 128
    assert length % 128 == 0 and 3 * nbs <= 128
    small = pools['small']
    mmp = pools['mm_ps']

    xst = small.tile([3 * nbs, 128], F32, tag=f'xst{nbs}')
    src = bass.AP(pc_ap.tensor, pc_ap.offset,
                  [[128, nbs], [length, 3], [1, 128]])
    nc.sync.dma_start(xst[:], src)
    xsq = small.tile([3 * nbs, 128], F32, tag=f'xsq{nbs}')
    nc.scalar.activation(xsq[:], xst[:], AF.Square)
    ps = mmp.tile([128, 512], F32, tag='mm')
    nc.tensor.matmul(ps[:, :nbs], lhsT=xsq[:], rhs=blk3[:3 * nbs, :nbs],
                     start=True, stop=True)
    sq_sb = small.tile([128, nbs], F32, tag=f'sqsb{nbs}')
    nc.scalar.activation(sq_sb[:], ps[:, :nbs], AF.Copy)
    ps2 = mmp.tile([128, 512], F32, tag='mm')
    nc.tensor.matmul(ps2[:nbs, :128], lhsT=sq_sb[:], rhs=id128[:],
                     start=True, stop=True)
    negsq = small.tile([nbs, 128], F32, tag=f'negsq{nbs}')
    nc.scalar.activation(negsq[:], ps2[:nbs, :128], AF.Copy, scale=-1.0)
    scratch = dram_pool.tile([nbs, 128], F32, tag=tag)
    nc.scalar.dma_start(scratch[:], negsq[:])
    return scratch


def build_nc(Q=N // NCORES, NK=N, repeat=1):
    nc = bacc.Bacc("TRN2", num_devices=NCORES, debug=False)

    def din(name, shape, dt=F32):
        return nc.dram_tensor(name, list(shape), dt,
                              kind="ExternalInput").ap()

    ins = {}
    for nm, sh, dt in [
        ('knn1f', (B, C, NK), F32), ('knn2f', (B, C, NK), F32),
        ('feat1f16', (B, C, NK), BF16), ('feat2f16', (B, C, NK), BF16),
        ('pc1f', (B, 3, NK), F32), ('pc2f', (B, 3, NK), F32),
        ('pc1f16', (B, 3, NK), BF16), ('pc2f16', (B, 3, NK), BF16),
        ('knn1q', (B, C, Q), F32), ('knn2q', (B, C, Q), F32),
        ('feat1q16', (B, C, Q), BF16), ('feat2q16', (B, C, Q), BF16),
        ('pc1q', (B, 3, Q), F32), ('pc2q', (B, 3, Q), F32),
        ('pcn1q16', (B, 3, Q), BF16), ('pcn2q16', (B, 3, Q), BF16),
        ('wt11b', (C, C), BF16), ('wt22b', (C, C), BF16),
        ('wposb', (3, C), BF16),
        ('wm1r', (C, C), MLP_DT), ('wm2r', (C, C), MLP_DT),
        ('b22', (C, 1), F32), ('bqc11', (C, 1), F32),
        ('bm1', (C, 1), F32), ('bm2', (C, 1), F32),
        ('id128', (128, 128), F32), ('blk3', (128, 128), F32),
        ('ones_row', (1, NK), F32),
        ('rank8', (1, 8), U16), ('base32', (1, 32), U16),
    ]:
        ins[nm] = din(nm, sh, dt)
    out1 = nc.dram_tensor('out1', [B, C, Q], F32, kind="ExternalOutput").ap()
    out2 = nc.dram_tensor('out2', [B, C, Q], F32, kind="ExternalOutput").ap()

    with tile.TileContext(nc) as tc:
        _kernel_body(tc, ins, out1, out2, Q, NK, repeat)
    nc.compile()
    return nc


def _kernel_body(tc, ins, out1, out2, Q, NK, repeat=1):
    nc = tc.nc
    from contextlib import ExitStack
    ctx = ExitStack()
    NT = Q // 128
    NROW = 128 * NS

    pool = lambda name, bufs: ctx.enter_context(
        tc.tile_pool(name=name, bufs=bufs))
    consts = pool('consts', 1)
    small = pool('small', 2)
    kvp = pool('kv', 2)         # khat, akv
    kv1 = pool('kv1', 1)        # rinvrep, ssqall, augkv
    kblk = pool('kblk', 2)      # key-side block loads
    qp = pool('qtensors', 1)    # per-combo query tiles
    mlpp = pool('mlp', 2)
    sc_ps = ctx.enter_context(
        tc.tile_pool(name='sc_ps', bufs=3, space='PSUM'))
    mm_ps = ctx.enter_context(
        tc.tile_pool(name='mm_ps', bufs=2, space='PSUM'))
    dram_pool = ctx.enter_context(
        tc.tile_pool(name='dram', bufs=2, space='DRAM'))
    pools = {'small': small, 'mm_ps': mm_ps}

    def cload(name, shape, dt=F32):
        t = consts.tile(list(shape), dt, tag=name)
        nc.sync.dma_start(t[:], ins[name])
        return t

    id128 = cload('id128', (128, 128))
    blk3 = cload('blk3', (128, 128))
    wt11b = cload('wt11b', (C, C), BF16)
    wt22b = cload('wt22b', (C, C), BF16)
    wposb = cload('wposb', (3, C), BF16)
    wm1r = cload('wm1r', (C, C), MLP_DT)
    wm2r = cload('wm2r', (C, C), MLP_DT)
    b22 = cload('b22', (C, 1))
    bqc11 = cload('bqc11', (C, 1))
    bm1 = cload('bm1', (C, 1))
    bm2 = cload('bm2', (C, 1))
    eps128 = consts.tile([128, 1], F32, tag='eps128')
    nc.vector.memset(eps128[:], EPS)
    rank128 = consts.tile([128, 8], U16, tag='rank128')
    nc.sync.dma_start(
        rank128[:], bass.AP(ins['rank8'].tensor, ins['rank8'].offset,
                            [[0, 128], [1, 8]]))
    base32 = consts.tile([128, 32], U16, tag='base32')
    nc.sync.dma_start(
        base32[:], bass.AP(ins['base32'].tensor, ins['base32'].offset,
                           [[0, 128], [1, 32]]))
    ones16 = consts.tile([128, 32], I16, tag='ones16')
    nc.vector.memset(ones16[:], 1)

    # W_t11/bqc11 always on the query side, W_t22/b22 always on keys
    combos = []
    for bi in range(B):
        combos.append((out1, bi, 'knn1q', 'feat1q16', 'pc1q', 'pcn1q16',
                       'knn2f', 'feat2f16', 'pc2f', 'pc2f16'))
        combos.append((out2, bi, 'knn2q', 'feat2q16', 'pc2q', 'pcn2q16',
                       'knn1f', 'feat1f16', 'pc1f', 'pc1f16'))

    def prep_combo(cmb):
        (outap, bi, knnq_n, featq_n, pcq_n, pcnq_n,
         knnf_n, featf_n, pcf_n, pcf16_n) = cmb
        knnq_d = ins[knnq_n][bi]
        featq_d = ins[featq_n][bi]
        pcq_d = ins[pcq_n][bi]
        pcnq_d = ins[pcnq_n][bi]
        knnf_d = ins[knnf_n][bi]
        featf_d = ins[featf_n][bi]
        pcf_d = ins[pcf_n][bi]
        pcf16_d = ins[pcf16_n][bi]

        # ============== key-side prep ==============
        knnkv = kblk.tile([C, NK], F32, tag='knnkv', bufs=1)
        nc.sync.dma_start(knnkv[:], knnf_d)
        khat = kvp.tile([C, NK], F32, tag='khat')
        scr1 = dram_pool.tile([32, 128], F32, tag='scr1')
        for blk in range(NCH):
            sl = slice(blk * CHW, (blk + 1) * CHW)
            ssq = kblk.tile([C, CHW], F32, tag='ssqb', bufs=1)
            nc.scalar.activation(khat[:, sl], knnkv[:, sl], AF.Square)
            nc.gpsimd.partition_all_reduce(
                ssq[:], khat[:, sl], channels=C,
                reduce_op=bass_isa.ReduceOp.add)
            row = bass.AP(scr1[:].tensor, scr1[:].offset + blk * CHW,
                          [[CHW, 1], [1, CHW]])
            nc.scalar.dma_start(row, ssq[0:1, :])
        s32 = small.tile([32, 128], F32, tag='s32')
        nc.scalar.dma_start(s32[:], scr1[:])
        nrm = small.tile([32, 128], F32, tag='nrm32')
        nc.scalar.activation(nrm[:], s32[:], AF.Sqrt, bias=eps128[:32])
        rin = small.tile([32, 128], F32, tag='rin32')
        nc.vector.reciprocal(rin[:], nrm[:])
        scr2 = dram_pool.tile([32, 128], F32, tag='scr2')
        nc.scalar.dma_start(scr2[:], rin[:])
        rinvrep = kv1.tile([C, NK], F32, tag='rinvrep')
        nc.scalar.dma_start(
            rinvrep[:], bass.AP(scr2[:].tensor, scr2[:].offset,
                                [[0, C], [1, NK]]))
        # khat = knn * rinv  (Pool; khat currently holds sq)
        nc.gpsimd.tensor_tensor(khat[:], knnkv[:], rinvrep[:],
                                op=ALU.mult)

        # akv = Wkv@feat + b22 + Wpos@xyz  (bf16 inputs, fp32 psum)
        akv = kvp.tile([C, NK], F32, tag='akv')
        for kb8 in range(NK // 512):
            sl = slice(kb8 * 512, (kb8 + 1) * 512)
            fb = kblk.tile([C, 512], BF16, tag='fb16')
            nc.sync.dma_start(fb[:], featf_d[:, sl])
            xb = kblk.tile([3, 512], BF16, tag='xb16')
            nc.sync.dma_start(xb[:], pcf16_d[:, sl])
            ps = mm_ps.tile([C, 512], F32, tag='mm')
            nc.tensor.matmul(ps[:], lhsT=wt22b[:], rhs=fb[:],
                             start=True, stop=False)
            nc.tensor.matmul(ps[:], lhsT=wposb[:], rhs=xb[:],
                             start=False, stop=True)
            nc.scalar.activation(akv[:, sl], ps[:], AF.Identity,
                                 bias=b22[:])

        # augkv [5, NK]: rows 0-2 pc, row 3 -|k|^2, row 4 ones
        negk = _neg_sq_row(tc, pools, pcf_d, NK, blk3, id128,
                           dram_pool, 'negk')
        augkv = kv1.tile([5, NK], F32, tag='augkv', bufs=2)
        nc.scalar.dma_start(augkv[0:3, :], pcf_d)
        nc.scalar.dma_start(
            augkv[3:4, :], bass.AP(negk[:].tensor, negk[:].offset,
                                   [[0, 1], [1, NK]]))
        nc.scalar.dma_start(augkv[4:5, :], ins['ones_row'][:, :NK])

        # ============== query-side prep ==============
        knnq = qp.tile([C, Q], F32, tag='knnq', bufs=2)
        nc.sync.dma_start(knnq[:], knnq_d)
        pcq = qp.tile([3, Q], F32, tag='pcq')
        nc.sync.dma_start(pcq[:], pcq_d)
        augq = qp.tile([5, Q], F32, tag='augq', bufs=2)
        nc.scalar.activation(augq[0:3, :], pcq[:], AF.Copy, scale=2.0)
        nc.scalar.dma_start(augq[3:4, :], ins['ones_row'][:, :Q])
        negq = _neg_sq_row(tc, pools, pcq_d, Q, blk3, id128,
                           dram_pool, 'negq')
        nc.scalar.dma_start(
            augq[4:5, :], bass.AP(negq[:].tensor, negq[:].offset,
                                  [[0, 1], [1, Q]]))
        # cq = Wq@featq + Wpos@(-xyz_q) + b11 + bpos
        fq16 = qp.tile([C, Q], BF16, tag='fq16')
        nc.sync.dma_start(fq16[:], featq_d)
        xnq = qp.tile([3, Q], BF16, tag='xnq')
        nc.sync.dma_start(xnq[:], pcnq_d)
        cq = qp.tile([C, Q], F32, tag='cq', bufs=2)
        cps = mm_ps.tile([128, 512], F32, tag='mm')
        nc.tensor.matmul(cps[:C, :Q], lhsT=wt11b[:],
                         rhs=fq16[:], start=True, stop=False)
        nc.tensor.matmul(cps[:C, :Q], lhsT=wposb[:], rhs=xnq[:],
                         start=False, stop=True)
        nc.scalar.activation(cq[:], cps[:C, :Q], AF.Identity,
                             bias=bqc11[:])
        return dict(outap=outap, bi=bi, khat=khat, akv=akv, augkv=augkv,
                    knnq=knnq, augq=augq, cq=cq)

    def tiles_combo(st, flush=True):
        outap, bi = st['outap'], st['bi']
        khat, akv, augkv = st['khat'], st['akv'], st['augkv']
        knnq, augq, cq = st['knnq'], st['augq'], st['cq']

        def emit_pending():
            if tc._pend is None:
                return
            h2p, outapp, bip, tslp = tc._pend
            tc._pend = None
            pooled = small.tile([C, 128], BF16, tag='pooled')
            nc.vector.tensor_reduce(
                pooled[:], h2p[:].rearrange('c (q k) -> c q k', k=NS),
                axis=AXX, op=ALU.max)
            outf = small.tile([C, 128], F32, tag='outf')
            nc.scalar.activation(outf[:], pooled[:], AF.Copy)
            nc.scalar.dma_start(outapp[bip][:, tslp], outf[:])

        for t in range(NT):
            tsl = slice(t * 128, (t + 1) * 128)
            idx16 = small.tile([128, 16], U16, tag='idx16')
            mstate = []
            for mi, (lhs_ap, rhs_tile) in enumerate((
                    (knnq[:, tsl], khat),
                    (augq[:5, tsl], augkv))):
                candv = small.tile([128, 32], F32, tag='candv')
                candi = small.tile([128, 32], U16, tag='candi')
                for cki in range(NCH):
                    ps = sc_ps.tile([128, CHW], F32, tag='sc')
                    for half in range(2):
                        hsl = slice(cki * CHW + half * 512,
                                    cki * CHW + (half + 1) * 512)
                        nc.tensor.matmul(
                            ps[:, half * 512:(half + 1) * 512],
                            lhsT=lhs_ap, rhs=rhs_tile[:, hsl],
                            start=True, stop=True)
                    c8 = slice(cki * 8, (cki + 1) * 8)
                    nc.vector.max(candv[:, c8], ps[:])
                    nc.vector.max_index(candi[:, c8], candv[:, c8],
                                        ps[:])
                # launch the Pool rank-scatter early; defer its DVE
                # consumers so the round-trip hides under the next scans
                candg = small.tile([128, 32], U16, tag='candg')
                nc.vector.tensor_tensor(
                    candg[:], candi[:], base32[:], op=ALU.add)
                vals8 = small.tile([128, 8], F32, tag='vals8')
                nc.vector.max(vals8[:], candv[:])
                pos8 = small.tile([128, 8], U16, tag='pos8')
                nc.vector.max_index(pos8[:], vals8[:], candv[:])
                tmp32 = small.tile([128, 32], U16, tag='tmp32')
                nc.gpsimd.local_scatter(tmp32[:], rank128[:],
                                        pos8[:].bitcast(I16),
                                        channels=128, num_elems=32,
                                        num_idxs=8)
                mstate.append((candg, tmp32))
                if mi == 0:
                    emit_pending()   # previous tile maxpool as DVE filler
            for mi, (candg, tmp32) in enumerate(mstate):
                tmp2 = small.tile([128, 32], I16, tag='tmp2')
                nc.vector.tensor_tensor(
                    tmp2[:], tmp32[:].bitcast(I16),
                    ones16[:], op=ALU.subtract)
                nc.gpsimd.local_scatter(idx16[:, mi * 8:(mi + 1) * 8],
                                        candg[:], tmp2[:],
                                        channels=128, num_elems=8,
                                        num_idxs=32)

            # --- index transpose to gather layout ---
            idxf = small.tile([128, 16], F32, tag='idxf')
            nc.vector.tensor_copy(idxf[:], idx16[:].bitcast(I16))
            pst = mm_ps.tile([128, 512], F32, tag='mm')
            nc.tensor.matmul(pst[:16, :128], lhsT=idxf[:], rhs=id128[:],
                             start=True, stop=True)
            idxT = small.tile([C, 128], I16, tag='idxT')
            nc.scalar.activation(idxT[0:16, :], pst[:16, :128], AF.Copy)
            nc.scalar.dma_start(idxT[16:32, :], idxT[0:16, :])
            nc.scalar.dma_start(idxT[32:64, :], idxT[0:32, :])

            # --- gather neighbors (GpSimd) + add cq (Pool) ---
            ag = mlpp.tile([C, NROW], F32, tag='ag')
            nc.gpsimd.ap_gather(ag[:], akv[:], idxT[:], channels=C,
                                num_elems=NK, d=1, num_idxs=NROW)
            cq_b = cq[:, tsl].to_broadcast([C, 128, NS])
            nc.gpsimd.tensor_tensor(
                ag[:].rearrange('c (q k) -> c q k', k=NS),
                ag[:].rearrange('c (q k) -> c q k', k=NS),
                cq_b, op=ALU.add)
            n1 = mlpp.tile([C, NROW], MLP_DT, tag='mlpbuf')
            nc.scalar.activation(n1[:], ag[:], AF.Prelu, alpha=LEAKY)

            # --- layers 1, 2 ---
            h1 = mlpp.tile([C, NROW], MLP_DT, tag='mlpbuf')
            for j in range(NROW // 512):
                sl = slice(j * 512, (j + 1) * 512)
                ps = mm_ps.tile([128, 512], F32, tag='mm')
                nc.tensor.matmul(ps[:C, :], lhsT=wm1r[:], rhs=n1[:, sl],
                                 start=True, stop=True)
                nc.scalar.activation(h1[:, sl], ps[:C, :], AF.Prelu,
                                     bias=bm1[:], alpha=LEAKY)
            h2 = mlpp.tile([C, NROW], BF16, tag='h2', bufs=2)
            for j in range(NROW // 512):
                sl = slice(j * 512, (j + 1) * 512)
                ps = mm_ps.tile([128, 512], F32, tag='mm')
                nc.tensor.matmul(ps[:C, :], lhsT=wm2r[:], rhs=h1[:, sl],
                                 start=True, stop=True)
                nc.scalar.activation(h2[:, sl], ps[:C, :], AF.Prelu,
                                     bias=bm2[:], alpha=LEAKY)
            tc._pend = (h2, outap, bi, tsl)
        if flush:
            emit_pending()

    tc._pend = None
    nrun = repeat * len(combos)
    st = prep_combo(combos[0])
    for i in range(nrun):
        nxt = None
        if i + 1 < nrun:
            nxt = prep_combo(combos[(i + 1) % len(combos)])
        tiles_combo(st, flush=(i + 1 == nrun))
        st = nxt
    ctx.close()


# ======================= host side =======================

_CACHED = {}


def _get_nc(repeat=1):
    key = f'nc{repeat}'
    if key not in _CACHED:
        _CACHED[key] = build_nc(repeat=repeat)
    return _CACHED[key]


def _round_tf32(x):
    u = np.ascontiguousarray(x, np.float32).view(np.uint32)
    r = (u + 0x00000FFF + ((u >> 13) & 1)) & 0xFFFFE000
    return r.view(np.float32)


def make_in_maps(pc1, pc2, feat1, feat2, knn1, knn2,
                 W_t11, b_t11, W_t22, b_t22, W_pos, b_pos,
                 W_m1, b_m1, W_m2, b_m2, Q=N // NCORES, NK=N,
                 ncores=NCORES):
    f32 = np.float32
    bf16 = mybir.dt.np(BF16)
    a = lambda x: np.ascontiguousarray(x, f32)
    a16 = lambda x: np.ascontiguousarray(
        np.asarray(x, f32).astype(bf16))
    base = {
        'knn1f': a(knn1), 'knn2f': a(knn2),
        'feat1f16': a16(feat1), 'feat2f16': a16(feat2),
        'pc1f': a(pc1), 'pc2f': a(pc2),
        'pc1f16': a16(pc1), 'pc2f16': a16(pc2),
        'wt11b': a16(np.asarray(W_t11).T),
        'wt22b': a16(np.asarray(W_t22).T),
        'wposb': a16(np.asarray(W_pos).T),
        'wm1r': _round_tf32(np.asarray(W_m1).T),
        'wm2r': _round_tf32(np.asarray(W_m2).T),
        'b22': a(np.asarray(b_t22).reshape(C, 1)),
        'bqc11': a((np.asarray(b_t11) + np.asarray(b_pos)).reshape(C, 1)),
        'bm1': a(np.asarray(b_m1).reshape(C, 1)),
        'bm2': a(np.asarray(b_m2).reshape(C, 1)),
        'id128': np.eye(128, dtype=f32),
        'blk3': (np.arange(128)[:, None] // 3 == np.arange(128)[None, :]
                 ).astype(f32),
        'ones_row': np.ones((1, NK), f32),
        'rank8': np.arange(1, 9, dtype=np.uint16).reshape(1, 8),
        'base32': (np.repeat(np.arange(4) * CHW, 8)
                   ).astype(np.uint16).reshape(1, 32),
    }
    pcn1 = a16(-np.asarray(pc1, f32))
    pcn2 = a16(-np.asarray(pc2, f32))
    in_maps = []
    for c in range(ncores):
        sl = slice(c * Q, (c + 1) * Q)
        m = dict(base)
        m['knn1q'] = a(base['knn1f'][:, :, sl])
        m['knn2q'] = a(base['knn2f'][:, :, sl])
        m['feat1q16'] = np.ascontiguousarray(base['feat1f16'][:, :, sl])
        m['feat2q16'] = np.ascontiguousarray(base['feat2f16'][:, :, sl])
        m['pc1q'] = a(base['pc1f'][:, :, sl])
        m['pc2q'] = a(base['pc2f'][:, :, sl])
        m['pcn1q16'] = np.ascontiguousarray(pcn1[:, :, sl])
        m['pcn2q16'] = np.ascontiguousarray(pcn2[:, :, sl])
        in_maps.append(m)
    return in_maps


def kernel(pc1, pc2, feat1, feat2, knn1, knn2,
           W_t11, b_t11, W_t22, b_t22, W_pos, b_pos,
           W_m1, b_m1, W_m2, b_m2):
    from concourse.bass_utils import run_bass_kernel_spmd
    nc = _get_nc()
    in_maps = make_in_maps(pc1, pc2, feat1, feat2, knn1, knn2,
                           W_t11, b_t11, W_t22, b_t22, W_pos, b_pos,
                           W_m1, b_m1, W_m2, b_m2)
    res = run_bass_kernel_spmd(nc, in_maps, core_ids=list(range(NCORES)))
    out1 = np.concatenate([res.results[c]['out1'] for c in range(NCORES)],
                          axis=2)
    out2 = np.concatenate([res.results[c]['out2'] for c in range(NCORES)],
                          axis=2)
    return out1, out2
